# revision 3
# baseline (speedup 1.0000x reference)
"""GCN 2-layer encoder on 8 Trainium2 NeuronCores (Bass/Tile).

Push-sharded design: core c owns src-slice [25000c, 25000(c+1)).
 - Per layer, each core builds a local node table T = (h * dinv) @ W  (f16,
   256B-stride rows in DRAM) for its 25088 padded local nodes.
 - Every edge (j -> i) is processed on owner(j): ELL slot layout grouped by
   (dst-owner, dst mod 4, per-core-degree desc); dma_gather pulls one table
   row per slot (<=8192 idx/call), strided tensor_reduce sums each node's
   slots, dma_scatter_add unpermutes partial sums into a natural-order
   partials tensor, and a ReduceScatter sums partials across the 8 cores.
 - Self-loops are ordinary slots; out_i = dinv_i * (sum slots) + b.
h1 = relu(out_L1); output = out_L2.
"""
import numpy as np

import concourse.bass as bass
import concourse.bacc as bacc
import concourse.mybir as mybir
import concourse.tile as tile
from concourse.bass_utils import run_bass_kernel_spmd
from concourse.masks import make_identity

N = 200000
E = 6400000
F_IN, F_HID, F_OUT = 32, 32, 16
NC = 8
SL = N // NC              # 25000 nodes per owner slice
LP = 25088                # padded local rows (196 * 128)
ZROW = LP                 # zero row id in table
TROWS = LP + 64           # table rows (64 zero rows)
NT = LP // 128            # 196 tiles per core
CLS = 4                   # dst classes (dst local % 4)
NPC = SL // CLS           # 6250 nodes per (owner, class)
NG = (NPC + 127) // 128   # 49 groups per (owner, class)
GPAD = NG * 128           # 6272
NBLK = NC * CLS           # 32 scatter blocks
MAXW = 64                 # max gather-chunk columns (8192 idx cap)
KMENU = np.array([1, 2, 3, 4, 5, 6, 7, 8, 10, 12, 14, 16, 20, 24, 28, 32,
                  40, 48, 56, 64, 80, 96, 112, 128], dtype=np.int64)

f32, f16, i16 = mybir.dt.float32, mybir.dt.float16, mybir.dt.int16


def _quantize_k(k):
    return KMENU[np.searchsorted(KMENU, np.maximum(k, 1))]


def my_dma_gather(nc, out_ap, in_ap, idxs_ap, num_idxs, elem_size, elem_step):
    eng = nc.gpsimd
    stride_bytes = elem_step * mybir.dt.size(in_ap.dtype)
    assert stride_bytes % 256 == 0 and num_idxs <= 8192
    _in_ap = eng.lower_ap_dma(in_ap, for_custom_bir_dma=True)
    return eng.add_instruction(
        mybir.InstDMAGatherAnt(
            name=nc.get_next_instruction_name(),
            ins=[*_in_ap, eng.lower_ap(idxs_ap),
                 eng.lower_val_access(eng.to_reg(num_idxs))],
            outs=[eng.lower_ap(out_ap)],
            transpose=False, num_idxs=num_idxs, elem_size=elem_size,
            stride_bytes_256=stride_bytes // 256, gen_mode=0,
            single_packet=False, queue_num=0, sbuf_tokens_per_rank=0,
            sbuf_free_dim_per_rank=0, sbuf_free_dim_pad_per_rank=0,
            sbuf_byte_offset=0,
        ))


def my_dma_scatter_add(nc, out_ap, in_ap, idxs_ap, num_idxs, elem_size, elem_step):
    eng = nc.gpsimd
    stride_bytes = elem_step * mybir.dt.size(out_ap.dtype)
    assert stride_bytes % 256 == 0 and num_idxs <= 8192
    _out_ap = eng.lower_ap_dma(out_ap, for_custom_bir_dma=True)
    return eng.add_instruction(
        mybir.InstDMAScatterAddAnt(
            name=nc.get_next_instruction_name(),
            ins=[eng.lower_ap(in_ap), eng.lower_ap(idxs_ap),
                 eng.lower_val_access(eng.to_reg(num_idxs))],
            outs=[*_out_ap],
            num_idxs=num_idxs, elem_size=elem_size,
            stride_bytes_256=stride_bytes // 256, read_from_swizzled=False,
            gen_mode=0, single_packet=False, queue_num=0,
            sbuf_tokens_per_rank=0,
        ))


def _rep16(stream):
    """Lay out an index stream [S] as the dma_gather/scatter idx tile
    [128, S/16]: idx j -> partition j%16, col j//16, replicated x8."""
    S = stream.shape[0]
    assert S % 16 == 0
    return np.tile(stream.reshape(S // 16, 16).T, (8, 1)).astype(np.int16)


def prepare(x, edge_index, W1, b1, W2, b2):
    """Host-side sharding: per-core inputs + common static schedule."""
    src = np.asarray(edge_index[0], dtype=np.int64)
    dst = np.asarray(edge_index[1], dtype=np.int64)
    x = np.asarray(x)
    deg = (np.bincount(dst, minlength=N) + 1).astype(np.float32)

    owner = src // SL
    o1 = np.argsort(owner * np.int64(N) + dst, kind="stable")
    s_s, d_s, o_s = src[o1], dst[o1], owner[o1]
    bounds = np.searchsorted(o_s, np.arange(NC + 1) * SL, side="left")
    # owner values are 0..7; searchsorted on o_s*SL trick wrong; recompute:
    bounds = np.searchsorted(o_s, np.arange(NC + 1), side="left")

    # per-core CSR over all N dst (edges + self loops)
    cores = []
    k_per_core = np.zeros((NC, NBLK, NG), dtype=np.int64)
    for c in range(NC):
        a, b = bounds[c], bounds[c + 1]
        es = s_s[a:b] - SL * c          # local src
        ed = d_s[a:b]                   # global dst (sorted)
        own_ids = np.arange(SL, dtype=np.int64)
        all_d = np.concatenate([ed, own_ids + SL * c])
        all_s = np.concatenate([es, own_ids]).astype(np.int64)
        o2 = np.argsort(all_d, kind="stable")
        all_s = all_s[o2]
        counts = np.bincount(all_d, minlength=N).astype(np.int64)
        ptr = np.zeros(N + 1, dtype=np.int64)
        np.cumsum(counts, out=ptr[1:])

        # node ordering per (dst-owner, class): degree desc, dummies last
        order = np.full((NBLK, GPAD), -1, dtype=np.int64)
        for o in range(NC):
            for cl in range(CLS):
                ids = SL * o + CLS * np.arange(NPC, dtype=np.int64) + cl
                srt = ids[np.argsort(-counts[ids], kind="stable")]
                order[o * CLS + cl, :NPC] = srt
        n_ord = np.where(order >= 0, counts[np.maximum(order, 0)], 0)
        k_per_core[c] = _quantize_k(
            n_ord.reshape(NBLK, NG, 128).max(axis=2))
        cores.append(dict(all_s=all_s, ptr=ptr, counts=counts, order=order))

    k_common = k_per_core.max(axis=0)           # [NBLK, NG]
    # chunk layout: per block, greedy group packing into <=MAXW columns
    blocks = []     # per block: list of chunks; chunk = list of (g, K)
    col_off = np.zeros((NBLK, NG), dtype=np.int64)
    for blk in range(NBLK):
        chunks, cur, w = [], [], 0
        for g in range(NG):
            K = int(k_common[blk, g])
            if w + K > MAXW:
                chunks.append(cur)
                cur, w = [], 0
            col_off[blk, g] = w
            cur.append((g, K))
            w += K
        chunks.append(cur)
        blocks.append(chunks)

    schedule = dict(k_common=k_common, blocks=blocks)

    in_maps = []
    for c in range(NC):
        cc = cores[c]
        gather_parts = []
        sidx1 = np.empty((NBLK, GPAD), dtype=np.int64)
        sidx2 = np.empty((NBLK, GPAD), dtype=np.int64)
        for blk in range(NBLK):
            ordb = cc["order"][blk]                       # [6272]
            loc = ordb - (ordb // SL) * SL                # local natural row
            sidx1[blk] = np.where(ordb >= 0, loc >> 1, -1)
            sidx2[blk] = np.where(ordb >= 0, loc >> 2, -1)
            nb = np.where(ordb >= 0, cc["counts"][np.maximum(ordb, 0)], 0)
            pb = np.where(ordb >= 0, cc["ptr"][np.maximum(ordb, 0)], 0)
            for chunk in blocks[blk]:
                for (g, K) in chunk:
                    nodes = slice(g * 128, (g + 1) * 128)
                    cols = np.arange(K)[None, :]
                    take = pb[nodes][:, None] + cols
                    valid = cols < nb[nodes][:, None]
                    vals = np.where(
                        valid, cc["all_s"][np.clip(take, 0, len(cc["all_s"]) - 1)],
                        ZROW)
                    # slot (g,k,p): stream col-major = vals.T [K,128]
                    gather_parts.append(vals.T.ravel())
        gstream = np.concatenate(gather_parts)
        xc = np.zeros((LP, F_IN), dtype=np.float32)
        xc[:SL] = x[SL * c: SL * (c + 1)]
        degc = np.ones((LP,), dtype=np.float32)
        degc[:SL] = deg[SL * c: SL * (c + 1)]
        in_maps.append({
            "xc": xc,
            "degc": degc.reshape(NT, 128).T.copy(),
            "w1": np.asarray(W1, np.float32), "w2": np.asarray(W2, np.float32),
            "b1": np.asarray(b1, np.float32).reshape(1, F_HID),
            "b2": np.asarray(b2, np.float32).reshape(1, F_OUT),
            "gidx": _rep16(gstream.astype(np.int16)),
            "sidx1": _rep16(sidx1.ravel().astype(np.int16)),
            "sidx2": _rep16(sidx2.ravel().astype(np.int16)),
        })
    return in_maps, schedule


def build(schedule):
    blocks = schedule["blocks"]
    swcols = sum(K for chunks in blocks for ch in chunks for (_, K) in ch)
    nc = bacc.Bacc("TRN2", target_bir_lowering=False, debug=False,
                   num_devices=NC)
    xc = nc.dram_tensor("xc", [LP, F_IN], f32, kind="ExternalInput").ap()
    degc = nc.dram_tensor("degc", [128, NT], f32, kind="ExternalInput").ap()
    w1 = nc.dram_tensor("w1", [F_IN, F_HID], f32, kind="ExternalInput").ap()
    w2 = nc.dram_tensor("w2", [F_HID, F_OUT], f32, kind="ExternalInput").ap()
    b1 = nc.dram_tensor("b1", [1, F_HID], f32, kind="ExternalInput").ap()
    b2 = nc.dram_tensor("b2", [1, F_OUT], f32, kind="ExternalInput").ap()
    gidx = nc.dram_tensor("gidx", [128, swcols * 8], i16, kind="ExternalInput").ap()
    sidx1 = nc.dram_tensor("sidx1", [128, NBLK * GPAD // 16], i16, kind="ExternalInput").ap()
    sidx2 = nc.dram_tensor("sidx2", [128, NBLK * GPAD // 16], i16, kind="ExternalInput").ap()
    out = nc.dram_tensor("out", [SL, F_OUT], f32, kind="ExternalOutput").ap()

    tab = nc.dram_tensor("tab", [TROWS, 128], f16).ap()
    part1 = nc.dram_tensor("part1", [N // 2, 64], f32).ap()
    part2 = nc.dram_tensor("part2", [N // 4, 64], f32).ap()
    rs1 = nc.dram_tensor("rs1", [SL // 2, 64], f32).ap()
    rs2 = nc.dram_tensor("rs2", [SL // 4, 64], f32).ap()

    with tile.TileContext(nc) as tc:
        ctx = dict(locals())
        _build_body(nc, tc, ctx)
        ctx["_es"].close()
    nc.compile()
    return nc


def _build_body(nc, tc, t):
    blocks = t["blocks"]
    xc, degc, w1, w2, b1, b2 = t["xc"], t["degc"], t["w1"], t["w2"], t["b1"], t["b2"]
    gidx, sidx1, sidx2, out = t["gidx"], t["sidx1"], t["sidx2"], t["out"]
    tab, part1, part2, rs1, rs2 = t["tab"], t["part1"], t["part2"], t["rs1"], t["rs2"]

    from contextlib import ExitStack
    es = ExitStack()
    const = es.enter_context(tc.tile_pool(name="const", bufs=1))
    sbp = es.enter_context(tc.tile_pool(name="sb", bufs=3))
    msgp = es.enter_context(tc.tile_pool(name="msg", bufs=4))
    redp = es.enter_context(tc.tile_pool(name="red", bufs=3))
    psp = es.enter_context(tc.tile_pool(name="ps", bufs=4, space="PSUM"))
    t["_es"] = es
    ident = const.tile([128, 128], f32)
    make_identity(nc, ident[:])
    w1t = const.tile([F_IN, F_HID], f32)
    nc.sync.dma_start(out=w1t[:], in_=w1[:])
    w2t = const.tile([F_HID, F_OUT], f32)
    nc.sync.dma_start(out=w2t[:], in_=w2[:])
    b1r = const.tile([1, F_HID], f32)
    nc.sync.dma_start(out=b1r[:], in_=b1[:])
    b1bc = const.tile([128, F_HID], f32)
    nc.gpsimd.partition_broadcast(b1bc[:], b1r[:])
    b2r = const.tile([1, F_OUT], f32)
    nc.sync.dma_start(out=b2r[:], in_=b2[:])
    b2bc = const.tile([128, F_OUT], f32)
    nc.gpsimd.partition_broadcast(b2bc[:], b2r[:])
    degt = const.tile([128, NT], f32)
    nc.sync.dma_start(out=degt[:], in_=degc[:])
    dsq = const.tile([128, NT], f32)
    nc.scalar.activation(dsq[:], degt[:], mybir.ActivationFunctionType.Sqrt)
    dinv = const.tile([128, NT], f32)
    nc.vector.reciprocal(dinv[:], dsq[:])

    # zero part1/part2 and tab zero-rows
    zt = const.tile([128, 4096], f32)
    nc.vector.memset(zt[:], 0)
    for dest, rows in ((part1, N // 2), (part2, N // 4)):
        flat = dest.rearrange("a b -> (a b)").rearrange("(p c) -> p c", p=128)
        ncols = rows * 64 // 128
        for s in range(0, ncols, 4096):
            w = min(4096, ncols - s)
            nc.sync.dma_start(out=flat[:, s:s + w], in_=zt[:, :w])
    ztab = const.tile([64, 128], f16)
    nc.vector.memset(ztab[:], 0)
    nc.sync.dma_start(out=tab[LP:TROWS, :], in_=ztab[:])

    def build_table(src_view, w_t, fo, scale_twice, relu, rows_valid):
        """tab[:, :fo] (f16) = scale(src) @ W per 128-row tile."""
        for g2 in range(NT):
            r0 = g2 * 128
            nrow = min(rows_valid - r0, 128)
            xt = sbp.tile([128, src_view.shape[1]], f32, tag="xt")
            if nrow < 128:
                nc.vector.memset(xt[:], 0)
            nc.sync.dma_start(out=xt[:nrow], in_=src_view[r0:r0 + nrow])
            h = sbp.tile([128, src_view.shape[1]], f32, tag="h")
            if relu:
                tmp = sbp.tile([128, src_view.shape[1]], f32, tag="tmp")
                nc.vector.tensor_scalar_mul(tmp[:], xt[:], dinv[:, g2:g2 + 1])
                nc.vector.tensor_add(tmp[:], tmp[:], b1bc[:])
                nc.scalar.activation(h[:], tmp[:],
                                     mybir.ActivationFunctionType.Relu)
                xs = sbp.tile([128, src_view.shape[1]], f32, tag="xs")
                nc.vector.tensor_scalar_mul(xs[:], h[:], dinv[:, g2:g2 + 1])
            else:
                xs = sbp.tile([128, src_view.shape[1]], f32, tag="xs")
                nc.vector.tensor_scalar_mul(xs[:], xt[:], dinv[:, g2:g2 + 1])
            tp = psp.tile([32, 128], f32, tag="tp")
            nc.tensor.transpose(tp[:], xs[:], ident[:])
            xT = sbp.tile([32, 128], f32, tag="xT")
            nc.vector.tensor_copy(xT[:], tp[:])
            hw = psp.tile([128, fo], f32, tag="hw")
            nc.tensor.matmul(hw[:], lhsT=xT[:], rhs=w_t[:], start=True, stop=True)
            st = sbp.tile([128, fo], f16, tag="st")
            nc.vector.tensor_copy(st[:], hw[:])
            nc.sync.dma_start(out=tab[r0:r0 + 128, 0:fo], in_=st[:])

    def push_layer(fo, part, sidx, rows_per_cl, colw):
        """gather + reduce + scatter for one layer. colw: scatter col width."""
        scol = 0  # running gather stream column
        for blk in range(NBLK):
            o, cl = blk // CLS, blk % CLS
            red = redp.tile([128, NG, fo], f32, tag="red")
            for chunk in blocks[blk]:
                wsum = sum(K for (_, K) in chunk)
                idxt = msgp.tile([128, MAXW * 8], i16, tag="gi")
                nc.sync.dma_start(out=idxt[:, :wsum * 8],
                                  in_=gidx[:, scol * 8:(scol + wsum) * 8])
                msg = msgp.tile([128, MAXW, fo], f16, tag="msg")
                my_dma_gather(nc, msg[:, :wsum, :], tab[:, 0:fo],
                              idxt[:, :wsum * 8], num_idxs=128 * wsum,
                              elem_size=fo, elem_step=128)
                # equal-K runs within chunk
                i = 0
                while i < len(chunk):
                    j = i
                    K = chunk[i][1]
                    while j < len(chunk) and chunk[j][1] == K:
                        j += 1
                    gcount = j - i
                    c0 = sum(KK for (_, KK) in chunk[:i])
                    g0 = chunk[i][0]
                    v = msg[:, c0:c0 + gcount * K, :].rearrange(
                        "p (g k) f -> p g f k", k=K)
                    nc.vector.tensor_reduce(
                        out=red[:, g0:g0 + gcount, :].rearrange("p g f -> p (g f)"),
                        in_=v, axis=mybir.AxisListType.X, op=mybir.AluOpType.add)
                    i = j
                scol += wsum
            sit = msgp.tile([128, GPAD // 16], i16, tag="si")
            nc.sync.dma_start(
                out=sit[:], in_=sidx[:, blk * (GPAD // 16):(blk + 1) * (GPAD // 16)])
            coff = colw * (cl if colw == F_OUT else (cl & 1))
            dest = part[rows_per_cl * o:rows_per_cl * (o + 1),
                        coff:coff + fo]
            my_dma_scatter_add(nc, dest, red[:], sit[:],
                               num_idxs=GPAD, elem_size=fo, elem_step=64)

    # ---- layer 1 ----
    build_table(xc, w1t, F_HID, False, False, LP)
    push_layer(F_HID, part1, sidx1, SL // 2, F_HID)
    nc.gpsimd.collective_compute(
        "ReduceScatter", mybir.AluOpType.add,
        replica_groups=[list(range(NC))], ins=[part1[:]], outs=[rs1[:]])
    # ---- layer 2 ----
    rs1v = rs1.rearrange("a (u b) -> (a u) b", u=2)       # [25000, 32]
    build_table(rs1v, w2t, F_OUT, True, True, SL)
    push_layer(F_OUT, part2, sidx2, SL // 4, F_OUT)
    nc.gpsimd.collective_compute(
        "ReduceScatter", mybir.AluOpType.add,
        replica_groups=[list(range(NC))], ins=[part2[:]], outs=[rs2[:]])
    # ---- final post ----
    rs2v = rs2.rearrange("a (u b) -> (a u) b", u=4)       # [25000, 16]
    for g2 in range(NT):
        r0 = g2 * 128
        nrow = min(SL - r0, 128)
        if nrow <= 0:
            break
        at = sbp.tile([128, F_OUT], f32, tag="at")
        nc.sync.dma_start(out=at[:nrow], in_=rs2v[r0:r0 + nrow])
        o1 = sbp.tile([128, F_OUT], f32, tag="o1")
        nc.vector.tensor_scalar_mul(o1[:], at[:], dinv[:, g2:g2 + 1])
        nc.vector.tensor_add(o1[:], o1[:], b2bc[:])
        nc.sync.dma_start(out=out[r0:r0 + nrow], in_=o1[:nrow])


_CACHE = {}


def kernel(x, edge_index, W1, b1, W2, b2):
    in_maps, schedule = prepare(x, edge_index, W1, b1, W2, b2)
    key = schedule["k_common"].tobytes()
    if key not in _CACHE:
        _CACHE[key] = build(schedule)
    nc = _CACHE[key]
    res = run_bass_kernel_spmd(nc, in_maps, list(range(NC)))
    return np.concatenate([res.results[c]["out"] for c in range(NC)], axis=0)


# revision 5
# speedup vs baseline: 120654582.0000x; 120654582.0000x over previous
"""GCN 2-layer encoder on 8 Trainium2 NeuronCores (Bass/Tile).

Push-sharded design: core c owns src-slice [25000c, 25000(c+1)).
 - Per layer, each core builds a local node table T = (h * dinv) @ W  (f16,
   256B-stride rows in DRAM) for its 25088 padded local nodes.
 - Every edge (j -> i) is processed on owner(j): ELL slot layout grouped by
   (dst-owner, dst mod 4, per-core-degree desc); dma_gather pulls one table
   row per slot (<=8192 idx/call), strided tensor_reduce sums each node's
   slots, dma_scatter_add unpermutes partial sums into a natural-order
   partials tensor, and a ReduceScatter sums partials across the 8 cores.
 - Self-loops are ordinary slots; out_i = dinv_i * (sum slots) + b.
h1 = relu(out_L1); output = out_L2.
"""
import numpy as np

import concourse.bass as bass
import concourse.bacc as bacc
import concourse.mybir as mybir
import concourse.tile as tile
from concourse.bass_utils import run_bass_kernel_spmd
from concourse.masks import make_identity

N = 200000
E = 6400000
F_IN, F_HID, F_OUT = 32, 32, 16
NC = 8
SL = N // NC              # 25000 nodes per owner slice
LP = 25088                # padded local rows (196 * 128)
ZROW = LP                 # zero row id in table
TROWS = LP + 64           # table rows (64 zero rows)
NT = LP // 128            # 196 tiles per core
CLS = 4                   # dst classes (dst local % 4)
NPC = SL // CLS           # 6250 nodes per (owner, class)
NG = (NPC + 127) // 128   # 49 groups per (owner, class)
GPAD = NG * 128           # 6272
NBLK = NC * CLS           # 32 scatter blocks
MAXW = 64                 # max gather-chunk columns (8192 idx cap)
KMENU = np.array([1, 2, 3, 4, 5, 6, 7, 8, 10, 12, 14, 16, 20, 24, 28, 32,
                  40, 48, 56, 64, 80, 96, 112, 128], dtype=np.int64)

f32, f16, i16 = mybir.dt.float32, mybir.dt.float16, mybir.dt.int16


def _quantize_k(k):
    return KMENU[np.searchsorted(KMENU, np.maximum(k, 1))]


def my_dma_gather(nc, out_ap, in_ap, idxs_ap, num_idxs, elem_size, elem_step):
    eng = nc.gpsimd
    stride_bytes = elem_step * mybir.dt.size(in_ap.dtype)
    assert stride_bytes % 256 == 0 and num_idxs <= 8192
    _in_ap = eng.lower_ap_dma(in_ap, for_custom_bir_dma=True)
    return eng.add_instruction(
        mybir.InstDMAGatherAnt(
            name=nc.get_next_instruction_name(),
            ins=[*_in_ap, eng.lower_ap(idxs_ap),
                 eng.lower_val_access(eng.to_reg(num_idxs))],
            outs=[eng.lower_ap(out_ap)],
            transpose=False, num_idxs=num_idxs, elem_size=elem_size,
            stride_bytes_256=stride_bytes // 256, gen_mode=0,
            single_packet=False, queue_num=0, sbuf_tokens_per_rank=0,
            sbuf_free_dim_per_rank=0, sbuf_free_dim_pad_per_rank=0,
            sbuf_byte_offset=0,
        ))


def my_dma_scatter_add(nc, out_ap, in_ap, idxs_ap, num_idxs, elem_size, elem_step):
    eng = nc.gpsimd
    stride_bytes = elem_step * mybir.dt.size(out_ap.dtype)
    assert stride_bytes % 256 == 0 and num_idxs <= 8192
    _out_ap = eng.lower_ap_dma(out_ap, for_custom_bir_dma=True)
    return eng.add_instruction(
        mybir.InstDMAScatterAddAnt(
            name=nc.get_next_instruction_name(),
            ins=[eng.lower_ap(in_ap), eng.lower_ap(idxs_ap),
                 eng.lower_val_access(eng.to_reg(num_idxs))],
            outs=[*_out_ap],
            num_idxs=num_idxs, elem_size=elem_size,
            stride_bytes_256=stride_bytes // 256, read_from_swizzled=False,
            gen_mode=0, single_packet=False, queue_num=0,
            sbuf_tokens_per_rank=0,
        ))


def _rep16(stream):
    """Lay out an index stream [S] as the dma_gather/scatter idx tile
    [128, S/16]: idx j -> partition j%16, col j//16, replicated x8."""
    S = stream.shape[0]
    assert S % 16 == 0
    return np.tile(stream.reshape(S // 16, 16).T, (8, 1)).astype(np.int16)


def prepare(x, edge_index, W1, b1, W2, b2):
    """Host-side sharding: per-core inputs + common static schedule."""
    src = np.asarray(edge_index[0], dtype=np.int64)
    dst = np.asarray(edge_index[1], dtype=np.int64)
    x = np.asarray(x)
    deg = (np.bincount(dst, minlength=N) + 1).astype(np.float32)

    owner = src // SL
    o1 = np.argsort(owner * np.int64(N) + dst, kind="stable")
    s_s, d_s, o_s = src[o1], dst[o1], owner[o1]
    bounds = np.searchsorted(o_s, np.arange(NC + 1) * SL, side="left")
    # owner values are 0..7; searchsorted on o_s*SL trick wrong; recompute:
    bounds = np.searchsorted(o_s, np.arange(NC + 1), side="left")

    # per-core CSR over all N dst (edges + self loops)
    cores = []
    k_per_core = np.zeros((NC, NBLK, NG), dtype=np.int64)
    for c in range(NC):
        a, b = bounds[c], bounds[c + 1]
        es = s_s[a:b] - SL * c          # local src
        ed = d_s[a:b]                   # global dst (sorted)
        own_ids = np.arange(SL, dtype=np.int64)
        all_d = np.concatenate([ed, own_ids + SL * c])
        all_s = np.concatenate([es, own_ids]).astype(np.int64)
        o2 = np.argsort(all_d, kind="stable")
        all_s = all_s[o2]
        counts = np.bincount(all_d, minlength=N).astype(np.int64)
        ptr = np.zeros(N + 1, dtype=np.int64)
        np.cumsum(counts, out=ptr[1:])

        # node ordering per (dst-owner, class): degree desc, dummies last
        order = np.full((NBLK, GPAD), -1, dtype=np.int64)
        for o in range(NC):
            for cl in range(CLS):
                ids = SL * o + CLS * np.arange(NPC, dtype=np.int64) + cl
                srt = ids[np.argsort(-counts[ids], kind="stable")]
                order[o * CLS + cl, :NPC] = srt
        n_ord = np.where(order >= 0, counts[np.maximum(order, 0)], 0)
        k_per_core[c] = _quantize_k(
            n_ord.reshape(NBLK, NG, 128).max(axis=2))
        cores.append(dict(all_s=all_s, ptr=ptr, counts=counts, order=order))

    k_common = k_per_core.max(axis=0)           # [NBLK, NG]
    # chunk layout: per block, greedy group packing into <=MAXW columns
    blocks = []     # per block: list of chunks; chunk = list of (g, K)
    col_off = np.zeros((NBLK, NG), dtype=np.int64)
    for blk in range(NBLK):
        chunks, cur, w = [], [], 0
        for g in range(NG):
            K = int(k_common[blk, g])
            if w + K > MAXW:
                chunks.append(cur)
                cur, w = [], 0
            col_off[blk, g] = w
            cur.append((g, K))
            w += K
        chunks.append(cur)
        blocks.append(chunks)

    schedule = dict(k_common=k_common, blocks=blocks)

    in_maps = []
    for c in range(NC):
        cc = cores[c]
        gather_parts = []
        sidx1 = np.empty((NBLK, GPAD), dtype=np.int64)
        sidx2 = np.empty((NBLK, GPAD), dtype=np.int64)
        for blk in range(NBLK):
            ordb = cc["order"][blk]                       # [6272]
            loc = ordb - (ordb // SL) * SL                # local natural row
            sidx1[blk] = np.where(ordb >= 0, loc >> 1, -1)
            sidx2[blk] = np.where(ordb >= 0, loc >> 2, -1)
            nb = np.where(ordb >= 0, cc["counts"][np.maximum(ordb, 0)], 0)
            pb = np.where(ordb >= 0, cc["ptr"][np.maximum(ordb, 0)], 0)
            for chunk in blocks[blk]:
                for (g, K) in chunk:
                    nodes = slice(g * 128, (g + 1) * 128)
                    cols = np.arange(K)[None, :]
                    take = pb[nodes][:, None] + cols
                    valid = cols < nb[nodes][:, None]
                    vals = np.where(
                        valid, cc["all_s"][np.clip(take, 0, len(cc["all_s"]) - 1)],
                        ZROW)
                    # slot (g,k,p): stream col-major = vals.T [K,128]
                    gather_parts.append(vals.T.ravel())
        gstream = np.concatenate(gather_parts)
        xc = np.zeros((LP, F_IN), dtype=np.float32)
        xc[:SL] = x[SL * c: SL * (c + 1)]
        degc = np.ones((LP,), dtype=np.float32)
        degc[:SL] = deg[SL * c: SL * (c + 1)]
        in_maps.append({
            "xc": xc,
            "degc": degc.reshape(NT, 128).T.copy(),
            "w1": np.asarray(W1, np.float32), "w2": np.asarray(W2, np.float32),
            "b1": np.asarray(b1, np.float32).reshape(1, F_HID),
            "b2": np.asarray(b2, np.float32).reshape(1, F_OUT),
            "gidx": _rep16(gstream.astype(np.int16)),
            "sidx1": _rep16(sidx1.ravel().astype(np.int16)),
            "sidx2": _rep16(sidx2.ravel().astype(np.int16)),
        })
    return in_maps, schedule


def build(schedule):
    blocks = schedule["blocks"]
    swcols = sum(K for chunks in blocks for ch in chunks for (_, K) in ch)
    nc = bacc.Bacc("TRN2", target_bir_lowering=False, debug=False,
                   num_devices=NC)
    xc = nc.dram_tensor("xc", [LP, F_IN], f32, kind="ExternalInput").ap()
    degc = nc.dram_tensor("degc", [128, NT], f32, kind="ExternalInput").ap()
    w1 = nc.dram_tensor("w1", [F_IN, F_HID], f32, kind="ExternalInput").ap()
    w2 = nc.dram_tensor("w2", [F_HID, F_OUT], f32, kind="ExternalInput").ap()
    b1 = nc.dram_tensor("b1", [1, F_HID], f32, kind="ExternalInput").ap()
    b2 = nc.dram_tensor("b2", [1, F_OUT], f32, kind="ExternalInput").ap()
    gidx = nc.dram_tensor("gidx", [128, swcols * 8], i16, kind="ExternalInput").ap()
    sidx1 = nc.dram_tensor("sidx1", [128, NBLK * GPAD // 16], i16, kind="ExternalInput").ap()
    sidx2 = nc.dram_tensor("sidx2", [128, NBLK * GPAD // 16], i16, kind="ExternalInput").ap()
    out = nc.dram_tensor("out", [SL, F_OUT], f32, kind="ExternalOutput").ap()

    tab = nc.dram_tensor("tab", [TROWS, 128], f16).ap()
    part1 = nc.dram_tensor("part1", [N // 2, 64], f32).ap()
    part2 = nc.dram_tensor("part2", [N // 4, 64], f32).ap()
    rs1 = nc.dram_tensor("rs1", [SL // 2, 64], f32).ap()
    rs2 = nc.dram_tensor("rs2", [SL // 4, 64], f32).ap()

    with tile.TileContext(nc) as tc:
        ctx = dict(locals())
        _build_body(nc, tc, ctx)
        ctx["_es"].close()
    nc.compile()
    return nc


def _build_body(nc, tc, t):
    blocks = t["blocks"]
    xc, degc, w1, w2, b1, b2 = t["xc"], t["degc"], t["w1"], t["w2"], t["b1"], t["b2"]
    gidx, sidx1, sidx2, out = t["gidx"], t["sidx1"], t["sidx2"], t["out"]
    tab, part1, part2, rs1, rs2 = t["tab"], t["part1"], t["part2"], t["rs1"], t["rs2"]

    from contextlib import ExitStack
    es = ExitStack()
    const = es.enter_context(tc.tile_pool(name="const", bufs=1))
    sbp = es.enter_context(tc.tile_pool(name="sb", bufs=4))
    msgp = es.enter_context(tc.tile_pool(name="msg", bufs=6))
    redp = es.enter_context(tc.tile_pool(name="red", bufs=4))
    psp = es.enter_context(tc.tile_pool(name="ps", bufs=4, space="PSUM"))
    t["_es"] = es
    ident = const.tile([128, 128], f32)
    make_identity(nc, ident[:])
    w1t = const.tile([F_IN, F_HID], f32)
    nc.sync.dma_start(out=w1t[:], in_=w1[:])
    w2t = const.tile([F_HID, F_OUT], f32)
    nc.sync.dma_start(out=w2t[:], in_=w2[:])
    b1r = const.tile([1, F_HID], f32)
    nc.sync.dma_start(out=b1r[:], in_=b1[:])
    b1bc = const.tile([128, F_HID], f32)
    nc.gpsimd.partition_broadcast(b1bc[:], b1r[:])
    b2r = const.tile([1, F_OUT], f32)
    nc.sync.dma_start(out=b2r[:], in_=b2[:])
    b2bc = const.tile([128, F_OUT], f32)
    nc.gpsimd.partition_broadcast(b2bc[:], b2r[:])
    degt = const.tile([128, NT], f32)
    nc.sync.dma_start(out=degt[:], in_=degc[:])
    dsq = const.tile([128, NT], f32)
    nc.scalar.activation(dsq[:], degt[:], mybir.ActivationFunctionType.Sqrt)
    dinv = const.tile([128, NT], f32)
    nc.vector.reciprocal(dinv[:], dsq[:])

    # zero part1/part2 and tab zero-rows
    zt = const.tile([128, 4096], f32)
    nc.vector.memset(zt[:], 0)
    for dest, rows in ((part1, N // 2), (part2, N // 4)):
        flat = dest.rearrange("a b -> (a b)").rearrange("(p c) -> p c", p=128)
        ncols = rows * 64 // 128
        for s in range(0, ncols, 4096):
            w = min(4096, ncols - s)
            nc.sync.dma_start(out=flat[:, s:s + w], in_=zt[:, :w])
    ztab = const.tile([64, 128], f16)
    nc.vector.memset(ztab[:], 0)
    nc.sync.dma_start(out=tab[LP:TROWS, :], in_=ztab[:])

    def build_table(src_view, w_t, fo, scale_twice, relu, rows_valid):
        """tab[:, :fo] (f16) = scale(src) @ W per 128-row tile."""
        for g2 in range(NT):
            r0 = g2 * 128
            nrow = min(rows_valid - r0, 128)
            xt = sbp.tile([128, src_view.shape[1]], f32, tag="xt")
            if nrow < 128:
                nc.vector.memset(xt[:], 0)
            nc.sync.dma_start(out=xt[:nrow], in_=src_view[r0:r0 + nrow])
            h = sbp.tile([128, src_view.shape[1]], f32, tag="h")
            if relu:
                tmp = sbp.tile([128, src_view.shape[1]], f32, tag="tmp")
                nc.vector.tensor_scalar_mul(tmp[:], xt[:], dinv[:, g2:g2 + 1])
                nc.vector.tensor_add(tmp[:], tmp[:], b1bc[:])
                nc.scalar.activation(h[:], tmp[:],
                                     mybir.ActivationFunctionType.Relu)
                xs = sbp.tile([128, src_view.shape[1]], f32, tag="xs")
                nc.vector.tensor_scalar_mul(xs[:], h[:], dinv[:, g2:g2 + 1])
            else:
                xs = sbp.tile([128, src_view.shape[1]], f32, tag="xs")
                nc.vector.tensor_scalar_mul(xs[:], xt[:], dinv[:, g2:g2 + 1])
            tp = psp.tile([32, 128], f32, tag="tp")
            nc.tensor.transpose(tp[:], xs[:], ident[:])
            xT = sbp.tile([32, 128], f32, tag="xT")
            nc.vector.tensor_copy(xT[:], tp[:])
            hw = psp.tile([128, fo], f32, tag="hw")
            nc.tensor.matmul(hw[:], lhsT=xT[:], rhs=w_t[:], start=True, stop=True)
            st = sbp.tile([128, fo], f16, tag="st")
            nc.vector.tensor_copy(st[:], hw[:])
            nc.sync.dma_start(out=tab[r0:r0 + 128, 0:fo], in_=st[:])

    def push_layer(fo, part, sidx, rows_per_cl, colw):
        """gather + reduce + scatter for one layer. colw: scatter col width."""
        scol = 0  # running gather stream column
        for blk in range(NBLK):
            o, cl = blk // CLS, blk % CLS
            red = redp.tile([128, NG, fo], f32, tag="red")
            for chunk in blocks[blk]:
                wsum = sum(K for (_, K) in chunk)
                idxt = msgp.tile([128, MAXW * 8], i16, tag="gi")
                nc.sync.dma_start(out=idxt[:, :wsum * 8],
                                  in_=gidx[:, scol * 8:(scol + wsum) * 8])
                msg = msgp.tile([128, MAXW, fo], f16, tag="msg")
                my_dma_gather(nc, msg[:, :wsum, :], tab[:, 0:fo],
                              idxt[:, :wsum * 8], num_idxs=128 * wsum,
                              elem_size=fo, elem_step=128)
                # equal-K runs within chunk
                i = 0
                while i < len(chunk):
                    j = i
                    K = chunk[i][1]
                    while j < len(chunk) and chunk[j][1] == K:
                        j += 1
                    gcount = j - i
                    c0 = sum(KK for (_, KK) in chunk[:i])
                    g0 = chunk[i][0]
                    v = msg[:, c0:c0 + gcount * K, :].rearrange(
                        "p (g k) f -> p g f k", k=K)
                    nc.vector.tensor_reduce(
                        out=red[:, g0:g0 + gcount, :].rearrange("p g f -> p (g f)"),
                        in_=v, axis=mybir.AxisListType.X, op=mybir.AluOpType.add)
                    i = j
                scol += wsum
            sit = msgp.tile([128, GPAD // 16], i16, tag="si")
            nc.sync.dma_start(
                out=sit[:], in_=sidx[:, blk * (GPAD // 16):(blk + 1) * (GPAD // 16)])
            coff = colw * (cl if colw == F_OUT else (cl & 1))
            dest = part[rows_per_cl * o:rows_per_cl * (o + 1),
                        coff:coff + fo]
            my_dma_scatter_add(nc, dest, red[:], sit[:],
                               num_idxs=GPAD, elem_size=fo, elem_step=64)

    # ---- layer 1 ----
    build_table(xc, w1t, F_HID, False, False, LP)
    push_layer(F_HID, part1, sidx1, SL // 2, F_HID)
    nc.gpsimd.collective_compute(
        "ReduceScatter", mybir.AluOpType.add,
        replica_groups=[list(range(NC))], ins=[part1[:]], outs=[rs1[:]])
    # ---- layer 2 ----
    rs1v = rs1.rearrange("a (u b) -> (a u) b", u=2)       # [25000, 32]
    build_table(rs1v, w2t, F_OUT, True, True, SL)
    push_layer(F_OUT, part2, sidx2, SL // 4, F_OUT)
    nc.gpsimd.collective_compute(
        "ReduceScatter", mybir.AluOpType.add,
        replica_groups=[list(range(NC))], ins=[part2[:]], outs=[rs2[:]])
    # ---- final post ----
    rs2v = rs2.rearrange("a (u b) -> (a u) b", u=4)       # [25000, 16]
    for g2 in range(NT):
        r0 = g2 * 128
        nrow = min(SL - r0, 128)
        if nrow <= 0:
            break
        at = sbp.tile([128, F_OUT], f32, tag="at")
        nc.sync.dma_start(out=at[:nrow], in_=rs2v[r0:r0 + nrow])
        o1 = sbp.tile([128, F_OUT], f32, tag="o1")
        nc.vector.tensor_scalar_mul(o1[:], at[:], dinv[:, g2:g2 + 1])
        nc.vector.tensor_add(o1[:], o1[:], b2bc[:])
        nc.sync.dma_start(out=out[r0:r0 + nrow], in_=o1[:nrow])


_CACHE = {}


def kernel(x, edge_index, W1, b1, W2, b2):
    in_maps, schedule = prepare(x, edge_index, W1, b1, W2, b2)
    key = schedule["k_common"].tobytes()
    if key not in _CACHE:
        _CACHE[key] = build(schedule)
    nc = _CACHE[key]
    res = run_bass_kernel_spmd(nc, in_maps, list(range(NC)))
    return np.concatenate([res.results[c]["out"] for c in range(NC)], axis=0)
